# revision 7
# baseline (speedup 1.0000x reference)
"""Trainium2 Bass kernel for a 2-layer GAT (GATConv x2 + linear head).

Strategy (8 NeuronCores, dst-node sharded, zero cross-core reduction):
  - Nodes are snake-dealt to 8 cores by in-degree (load balance); each core
    owns 12500 nodes (+44 pad ranks -> 12544 = 98 blocks of 128).
  - Global rank r = core*12544 + local. Node tables are indexed by rank.
  - Edges are grouped per (src-bucket b of 25088 ranks, dst-block of 128
    nodes). Within each (b, block), dst nodes are ordered by their bucket-b
    in-degree so the slot rectangle [128 nodes x k slots] is near-tight.
  - Gather: custom SWDGE dma_gather with int16 bucket-relative indices and
    relaxed element size (layer1 row = 16B: x(3)+a_s; layer2 row = 66B fp16:
    x2(32)+a_s2), table rows strided 256B. Gathers are batched ~8k indices
    per instruction (Pool-engine SWDGE desc-gen is the bottleneck at small
    sizes: ~2us fixed per instruction).
  - Per-edge softmax: z = a_s[src] + a_d[dst] (a_d is a per-partition column
    because dst == partition), Prelu+Exp on ACT, weight & segment-sum via an
    in-place multiply + strided free-dim tensor_reduce on DVE. The segment
    max subtraction is skipped (logits are in [-5, 5]; exp is safe and the
    softmax is shift-invariant).
  - Per-bucket partial sums accumulate in an SBUF tile [128, 98 blocks, W]
    (dst partition = node-within-block, in the bucket's degree-sorted perm
    order), stored once per bucket to a 256B-row DRAM table, then combined
    across buckets with 4 small SWDGE gathers (perm -> rank order) + DVE
    adds. No scatter-add, no DRAM zero-init.
  - W1/W2 are folded OUT of the tables (aggregation is linear in h): the
    tables carry raw features; W is applied once per layer at finalize via a
    PE transpose + block-diagonal-W matmul per 4 blocks.
  - Layer-2 node table is exchanged with a single AllGather (3.2MB/core).

kernel(**inputs) -> np.ndarray [100000, 1] float32.
"""

import numpy as np

import concourse.bass as bass
import concourse.mybir as mybir
import concourse.tile as tile
from concourse import bacc, ap_utils
from concourse._compat import exact_div
from concourse.bass_utils import run_bass_kernel_spmd

# ---------------------------------------------------------------- constants
N = 100000
E = 3200000
NC = 8
P = 128
NPC_REAL = 12500
NPC = 12544
NBLK = NPC // P            # 98
BUCKET = 2 * NPC           # 25088
NB = 4
NRANK = NC * NPC           # 100352
ROWF = 64                  # f32 table row stride in elems (256B)
ROWH = 128                 # fp16 table row stride in elems (256B)
L1W = 4                    # layer-1 gather width: x(3) + a_s1
L2W = 33                   # layer-2 gather width: x2(32) + a_s2
NEG = 0.2
A_S_PAD = -1.0e9
EPS = 1e-16
import os as _os
CAPS = int(_os.environ.get("GAT_CAPS", "63"))  # slots (x128 idxs) per gather
SBATCH = 4 * CAPS          # slots per compute batch (4 full pieces)
GSLAB = 4096               # gidx slab columns (int16) per load
DT = mybir.dt.float32
DH = mybir.dt.float16
DI = mybir.dt.int16


# ------------------------------------------------------- raw SWDGE gather
def dma_gather_raw(gp, out_ap, in_ap, idxs_ap, num_idxs, elem_size, elem_step,
                   queue_num=0):
    assert idxs_ap.dtype == DI
    assert in_ap.dtype == out_ap.dtype
    assert in_ap.space == bass.MemorySpace.DRAM
    assert ap_utils.ap_is_contiguous(out_ap.ap[1:])
    assert ap_utils.ap_is_contiguous(idxs_ap.ap[1:])
    assert in_ap.ap[-1][1] == out_ap.ap[-1][1] == elem_size
    assert out_ap.ap[0][1] * out_ap.ap[1][1] >= num_idxs
    assert in_ap.ap[0][0] == elem_step
    stride_bytes_256 = exact_div(elem_step * mybir.dt.size(in_ap.dtype), 256)
    assert 0 < stride_bytes_256 < 256
    _in_ap = gp.lower_ap_dma(in_ap, for_custom_bir_dma=True)
    _idxs_ap = gp.lower_ap(idxs_ap)
    _out_ap = gp.lower_ap(out_ap)
    return gp.add_instruction(
        mybir.InstDMAGatherAnt(
            name=gp.bass.get_next_instruction_name(),
            ins=[*_in_ap, _idxs_ap, gp.lower_val_access(gp.to_reg(num_idxs))],
            outs=[_out_ap],
            transpose=False,
            num_idxs=num_idxs,
            elem_size=elem_size,
            stride_bytes_256=stride_bytes_256,
            gen_mode=0,
            # single_packet coalesces each engine's descs into one packet;
            # packets cap at 64 descs / 4KB, i.e. 1024 idxs
            single_packet=num_idxs <= 1024,
            queue_num=queue_num,
            sbuf_tokens_per_rank=0,
            sbuf_free_dim_per_rank=0,
            sbuf_free_dim_pad_per_rank=0,
            sbuf_byte_offset=0,
        ))


def wrap16(idx):
    """[n] int -> SWDGE wrapped idx layout [128, n/16] int16 (8x replicated)."""
    n = len(idx)
    n16 = ((n + 15) // 16) * 16
    buf = np.full(n16, -1, np.int16)
    buf[:n] = idx
    w = buf.reshape(n16 // 16, 16).T
    return np.tile(w, (8, 1))


# ------------------------------------------------------- host preprocessing
def preprocess(edge_index):
    src = np.concatenate([edge_index[0].astype(np.int64),
                          np.arange(N, dtype=np.int64)])
    dst = np.concatenate([edge_index[1].astype(np.int64),
                          np.arange(N, dtype=np.int64)])

    deg = np.bincount(dst, minlength=N)
    order = np.argsort(-deg, kind="stable")
    pos = np.arange(N)
    rnd, lane = pos // NC, pos % NC
    core = np.where(rnd % 2 == 0, lane, NC - 1 - lane)
    node2rank = np.empty(N, np.int64)
    node2rank[order] = core * NPC + rnd

    srank = node2rank[src]
    drank = node2rank[dst]
    dcore = drank // NPC

    per_core = []
    counts = np.zeros((NC, NB, NPC), np.int64)
    for c in range(NC):
        m = dcore == c
        s_c, d_c = srank[m], drank[m] % NPC
        b_c = s_c // BUCKET
        per_core.append((s_c, d_c, b_c))
        for b in range(NB):
            mm = b_c == b
            counts[c, b] = np.bincount(d_c[mm], minlength=NPC)

    perms = np.empty((NC, NB, NPC), np.int64)
    for c in range(NC):
        for b in range(NB):
            perms[c, b] = np.argsort(-counts[c, b], kind="stable")

    # unified k per (bucket, block) across cores
    kk = np.zeros((NB, NBLK), np.int64)
    for b in range(NB):
        cnt = np.take_along_axis(counts[:, b], perms[:, b], axis=1)
        kk[b] = cnt.reshape(NC, NBLK, P).max(axis=(0, 2))

    # compute groups: consecutive same-k blocks, m*k <= SBATCH
    groups = []  # (b, g0, m, k)
    for b in range(NB):
        g = 0
        while g < NBLK:
            k = int(kk[b, g])
            if k == 0:
                g += 1
                continue
            mlim = max(1, SBATCH // k)
            m = 1
            while (m < mlim and g + m < NBLK and kk[b, g + m] == k):
                m += 1
            groups.append((b, g, m, k))
            g += m

    # sbatches: consecutive same-bucket groups, total slots <= SBATCH
    # each: dict(b, glist=[(g0, m, k, off)], S, pieces=[(col0, cols, tp, t0)])
    sbatches = []
    gi = 0
    while gi < len(groups):
        b = groups[gi][0]
        glist = []
        S = 0
        while gi < len(groups) and groups[gi][0] == b:
            _, g0, m, k = groups[gi]
            if S + m * k > SBATCH:
                break
            glist.append((g0, m, k, S))
            S += m * k
            gi += 1
        sbatches.append(dict(b=b, glist=glist, S=S))

    # per-core gather index streams, in sbatch/piece order
    gstream = [[] for _ in range(NC)]
    col = 0
    for sb in sbatches:
        b = sb["b"]
        rects = [np.full((sb["S"], P), NPC_REAL, np.int64)]
        for c in range(NC):
            s_c, d_c, b_c = per_core[c]
            mm = b_c == b
            sb_s, sb_d = s_c[mm], d_c[mm]
            o = np.argsort(sb_d, kind="stable")
            sb_s, sb_d = sb_s[o], sb_d[o]
            starts = np.searchsorted(sb_d, np.arange(NPC))
            ends = np.searchsorted(sb_d, np.arange(NPC) + 1)
            rect = np.full((sb["S"], P), NPC_REAL, np.int64)  # dummy row
            for (g0, m, k, off) in sb["glist"]:
                nodes = perms[c, b, g0 * P:(g0 + m) * P]
                for u in range(m):
                    nd = nodes[u * P:(u + 1) * P]
                    for p, nloc in enumerate(nd):
                        s0, s1 = starts[nloc], ends[nloc]
                        cnt = s1 - s0
                        row0 = off + u * k
                        vals = np.sort(sb_s[s0:s1] - BUCKET * b)
                        rect[row0:row0 + cnt, p] = vals
            rects.append(rect)
        # pieces of <= CAPS slots
        pieces = []
        t0 = 0
        while t0 < sb["S"]:
            tp = min(CAPS, sb["S"] - t0)
            pieces.append((col, tp * 8, tp, t0))
            for c in range(NC):
                part = rects[c + 1][t0:t0 + tp, :].reshape(-1)
                gstream[c].append(wrap16(part))
            col += tp * 8
            t0 += tp
        sb["pieces"] = pieces
    gidx_arr = [np.concatenate(gstream[c], axis=1) for c in range(NC)]

    # combine gather idxs: for final (p, g) -> position of node g*128+p in
    # perm_b order: pos = p_b*NBLK + g_b where perm_b[g_b*128+p_b] = node.
    # gather idx layout: idx[t*128 + p] = pos(node t*128+p).
    cstream = [[] for _ in range(NC)]
    meta_c = []   # per bucket: list of (col0, cols, tp, t0)
    ccol = 0
    for b in range(NB):
        pieces = []
        t0 = 0
        while t0 < NBLK:
            tp = min(CAPS, NBLK - t0)
            pieces.append((ccol, tp * 8, tp, t0))
            ccol += tp * 8
            t0 += tp
        meta_c.append(pieces)
    for c in range(NC):
        for b in range(NB):
            inv = np.empty(NPC, np.int64)
            inv[perms[c, b]] = np.arange(NPC)
            # perm position j = g_b*128 + p_b -> pos = p_b*NBLK + g_b
            pj = inv  # [node] -> j
            pos = (pj % P) * NBLK + (pj // P)
            # idx[t*128+p] = pos[t*128+p] (node local id = t*128+p)
            idxs = pos  # identity layout over local ids
            for (col0, cols, tp, t0) in meta_c[b]:
                part = idxs[t0 * P:(t0 + tp) * P]
                cstream[c].append(wrap16(part))
    cidx_arr = [np.concatenate(cstream[c], axis=1) for c in range(NC)]

    # a_d idx stream: per bucket, perm order (local ranks), pieces of CAPS
    adstream = [[] for _ in range(NC)]
    meta_ad = []
    acol = 0
    for b in range(NB):
        pieces = []
        t0 = 0
        while t0 < NBLK:
            tp = min(CAPS, NBLK - t0)
            pieces.append((acol, tp * 8, tp, t0))
            for c in range(NC):
                part = perms[c, b][t0 * P:(t0 + tp) * P]
                adstream[c].append(wrap16(part))
            acol += tp * 8
            t0 += tp
        meta_ad.append(pieces)
    adidx_arr = [np.concatenate(adstream[c], axis=1) for c in range(NC)]

    return dict(node2rank=node2rank, sbatches=sbatches, meta_c=meta_c,
                meta_ad=meta_ad, gidx=gidx_arr, cidx=cidx_arr,
                adidx=adidx_arr, perms=perms, gcols=col, ccols=ccol,
                adcols=acol)


# ------------------------------------------------------- program builder
def build_program(prep, weights):
    sbatches = prep["sbatches"]
    meta_c, meta_ad = prep["meta_c"], prep["meta_ad"]
    b1 = weights["b1"]; b2 = weights["b2"]
    bl = float(weights["bl"][0])
    if np.abs(b1).max() > 0 or np.abs(b2).max() > 0:
        raise NotImplementedError("nonzero b1/b2")

    nc = bacc.Bacc("TRN2", target_bir_lowering=False, debug=False,
                   enable_asserts=False, num_devices=NC,
                   num_swdge_queues=4,
                   dynamic_dma_scratch_size=32768)

    # ---- external tensors
    adc1 = nc.dram_tensor("adc1", [P, NB, NBLK], DT, kind="ExternalInput")
    gidx_d = nc.dram_tensor("gidx", [P, prep["gcols"]], DI, kind="ExternalInput")
    cidx_d = nc.dram_tensor("cidx", [P, prep["ccols"]], DI, kind="ExternalInput")
    adidx_d = nc.dram_tensor("adidx", [P, prep["adcols"]], DI, kind="ExternalInput")
    consts = nc.dram_tensor("consts", [P, 768], DT, kind="ExternalInput")
    # consts columns: 0:128 W1diag[12,128] (parts 0:12), 128:256 W2diag[128,128],
    # 256:288 vs2bc, 288:320 vd2bc, 320:352 Wlbc, 352:480 identity,
    # 640:738 padmaskh [128, NBLK] (0 / -30000 at pad ranks)
    y_d = nc.dram_tensor("y", [NPC, 1], DT, kind="ExternalOutput")

    # ---- internal DRAM
    tab1 = nc.dram_tensor("tab1", [NRANK, ROWF], DT, kind="ExternalInput")
    agin2 = nc.dram_tensor("agin2", [NPC, ROWH], DH)
    tab2 = nc.dram_tensor("tab2", [NRANK, ROWH], DH, addr_space="Shared")
    # per-bucket partial tables, 256B rows, row index = p*NBLK + g
    part_t = [[nc.dram_tensor(f"part{li}_{b}", [NPC, ROWH], DH)
               for b in range(NB)] for li in range(2)]

    with tile.TileContext(nc) as tc:
        with tc.tile_pool(name="const", bufs=1) as cpool, \
             tc.tile_pool(name="chunk", bufs=2) as chpool, \
             tc.tile_pool(name="small", bufs=3) as zpool, \
             tc.tile_pool(name="gix", bufs=2) as gixpool, \
             tc.tile_pool(name="pb", bufs=2) as pbpool, \
             tc.tile_pool(name="cg", bufs=2) as cgpool, \
             tc.tile_pool(name="psum", bufs=2, space="PSUM") as pspool:

            ct = cpool.tile([P, 768], DT)
            nc.sync.dma_start(ct[:], consts[:])
            W1diag = ct[:, 0:128]      # valid on partitions 0:12
            W2diag = ct[:, 128:256]
            vs2bc = ct[:, 256:288]
            vd2bc = ct[:, 288:320]
            Wlbc = ct[:, 320:352]
            ident = ct[:, 352:480]
            padmaskh = ct[:, 640:640 + NBLK]

            adcol1 = cpool.tile([P, NB, NBLK], DT, tag="adcol1")
            nc.sync.dma_start(adcol1[:], adc1[:])
            adcol2 = cpool.tile([P, NB, NBLK], DH, tag="adcol2")
            cixt = cpool.tile([P, prep["ccols"]], DI, tag="cixt")
            nc.sync.dma_start(cixt[:], cidx_d[:])
            adixt = cpool.tile([P, prep["adcols"]], DI, tag="adixt")
            nc.sync.dma_start(adixt[:], adidx_d[:])

            qrr = [0]

            def nextq():
                qrr[0] = (qrr[0] + 1) % 4
                return qrr[0]

            def edge_phase(li, tab, W, adcol_fn, dt_row=DT, estep=ROWF):
                slab = {"tile": None, "base": -1}

                def gix(col0, cols):
                    if (slab["tile"] is None or col0 < slab["base"]
                            or col0 + cols > slab["base"] + GSLAB):
                        t = gixpool.tile([P, GSLAB], DI, tag="gslab")
                        base = col0
                        csz = min(GSLAB, prep["gcols"] - base)
                        nc.sync.dma_start(t[:, 0:csz], gidx_d[:, base:base + csz])
                        slab["tile"], slab["base"] = t, base
                    b0 = col0 - slab["base"]
                    return slab["tile"][:, b0:b0 + cols]

                pb = None
                prev_b = -1

                def flush_pb(b):
                    # strided store: SBUF [P, NBLK, W+1] -> 256B rows
                    dest = part_t[li][b][:].rearrange(
                        "(p g) w -> p g w", p=P)[:, :, 0:W + 1]
                    nc.sync.dma_start(dest, pb[:])

                for sb in sbatches:
                    b = sb["b"]
                    if b != prev_b:
                        if pb is not None:
                            flush_pb(prev_b)
                        pb = pbpool.tile([P, NBLK, W + 1], DH, tag="pb")
                        nc.vector.memset(pb[:], 0.0)
                        prev_b = b
                    S = sb["S"]
                    chunk = chpool.tile([P, SBATCH, W], dt_row, tag="chunk")
                    for (col0, cols, tp, t0) in sb["pieces"]:
                        dma_gather_raw(
                            nc.gpsimd, chunk[:, t0:t0 + tp, :],
                            tab[BUCKET * b:BUCKET * (b + 1), 0:W],
                            gix(col0, cols), tp * P, W, estep,
                            queue_num=nextq())
                    # z = a_s + a_d per group; then Prelu+Exp on whole batch
                    z = zpool.tile([P, SBATCH], DT, tag="z")
                    ad = adcol_fn(b)
                    for (g0, m, k, off) in sb["glist"]:
                        nc.vector.tensor_tensor(
                            out=z[:, off:off + m * k].rearrange(
                                "p (m k) -> p m k", m=m),
                            in0=chunk[:, off:off + m * k, W - 1].rearrange(
                                "p (m k) -> p m k", m=m),
                            in1=ad[:, g0:g0 + m].rearrange(
                                "p (m o) -> p m o", o=1).to_broadcast([P, m, k]),
                            op=mybir.AluOpType.add)
                    zp = zpool.tile([P, SBATCH], DT, tag="zp")
                    nc.scalar.activation(zp[:, 0:S], z[:, 0:S],
                                         mybir.ActivationFunctionType.Prelu,
                                         alpha=NEG)
                    ex = zpool.tile([P, SBATCH], DT, tag="ex")
                    nc.scalar.activation(ex[:, 0:S], zp[:, 0:S],
                                         mybir.ActivationFunctionType.Exp)
                    nc.vector.tensor_tensor(
                        out=chunk[:, 0:S, 0:W - 1],
                        in0=chunk[:, 0:S, 0:W - 1],
                        in1=ex[:, 0:S].to_broadcast([P, S, W - 1]),
                        op=mybir.AluOpType.mult)
                    with nc.allow_low_precision(
                            reason="fp16 partials; DVE accumulates f32 "
                                   "internally, rel tol 2e-2"):
                        for (g0, m, k, off) in sb["glist"]:
                            nc.vector.tensor_reduce(
                                out=pb[:, g0:g0 + m, 0:W - 1],
                                in_=chunk[:, off:off + m * k, 0:W - 1].rearrange(
                                    "p (m k) w -> p m w k", m=m),
                                axis=mybir.AxisListType.X,
                                op=mybir.AluOpType.add)
                            nc.vector.tensor_reduce(
                                out=pb[:, g0:g0 + m, W - 1],
                                in_=ex[:, off:off + m * k].rearrange(
                                    "p (m k) -> p m k", m=m),
                                axis=mybir.AxisListType.X,
                                op=mybir.AluOpType.add)
                flush_pb(prev_b)

            def combine(li, W, spool):
                """4 perm->rank gathers + adds -> v [P, NBLK, W+1] f32."""
                v = spool.tile([P, NBLK, W + 1], DT, tag=f"v{li}")

                def cgather(b):
                    cg = cgpool.tile([P, NBLK, W + 1], DH, tag="cg")
                    for (col0, cols, tp, t0) in meta_c[b]:
                        dma_gather_raw(
                            nc.gpsimd, cg[:, t0:t0 + tp, :],
                            part_t[li][b][:, 0:W + 1],
                            cixt[:, col0:col0 + cols], tp * P, W + 1, ROWH,
                            queue_num=nextq())
                    return cg

                cg0, cg1 = cgather(0), cgather(1)
                nc.vector.tensor_tensor(out=v[:], in0=cg0[:], in1=cg1[:],
                                        op=mybir.AluOpType.add)
                cg2 = cgather(2)
                nc.vector.tensor_tensor(out=v[:], in0=v[:], in1=cg2[:],
                                        op=mybir.AluOpType.add)
                cg3 = cgather(3)
                nc.vector.tensor_tensor(out=v[:], in0=v[:], in1=cg3[:],
                                        op=mybir.AluOpType.add)
                return v

            # ---------------- layer 1
            edge_phase(0, tab1, L1W, lambda b: adcol1[:, b, :])

            f1pool = tc.tile_pool(name="f1", bufs=1)
            spool = f1pool.__enter__()
            v1 = combine(0, L1W, spool)
            rec1 = spool.tile([P, NBLK], DT, tag="rec1")
            nc.vector.tensor_scalar_add(rec1[:], v1[:, :, L1W - 1], EPS)
            nc.vector.reciprocal(rec1[:], rec1[:])
            vst1 = spool.tile([P, NBLK, 3], DT, tag="vst1")
            nc.vector.tensor_tensor(out=vst1[:], in0=v1[:, :, 0:3],
                                    in1=rec1[:].to_broadcast([P, NBLK, 3]),
                                    op=mybir.AluOpType.mult)
            st2 = spool.tile([P, NBLK, ROWH], DH, tag="st2")
            # W1 sandwich per 4-block unit
            for u in range(0, NBLK, 4):
                nu = min(4, NBLK - u)
                tp1 = pspool.tile([3 * nu, P], DT, space="PSUM", tag="tps")
                nc.tensor.transpose(
                    out=tp1[:],
                    in_=vst1[:, u:u + nu, :].rearrange("p a b -> p (a b)"),
                    identity=ident[:])
                t1s = zpool.tile([3 * nu, P], DT, tag="t1s")
                nc.vector.tensor_copy(out=t1s[:], in_=tp1[:])
                hp = pspool.tile([P, nu * 32], DT, space="PSUM", tag="hps")
                nc.tensor.matmul(hp[:], t1s[:], W1diag[0:3 * nu, 0:nu * 32],
                                 start=True, stop=True)
                nc.scalar.activation(
                    st2[:, u:u + nu, 0:32],
                    hp[:].rearrange("p (a b) -> p a b", a=nu),
                    mybir.ActivationFunctionType.Relu)
            # a_s2 / a_d2
            tmp2 = spool.tile([P, NBLK, 32], DT, tag="tmp2")
            asd = spool.tile([P, NBLK], DT, tag="asd")
            nc.vector.tensor_tensor(out=tmp2[:], in0=st2[:, :, 0:32],
                                    in1=vs2bc.rearrange("p (o w) -> p o w", o=1).to_broadcast([P, NBLK, 32]),
                                    op=mybir.AluOpType.mult)
            nc.vector.tensor_reduce(out=asd[:], in_=tmp2[:],
                                    axis=mybir.AxisListType.X,
                                    op=mybir.AluOpType.add)
            # pad-rank mask folded into the fp16 cast
            nc.vector.tensor_tensor(out=st2[:, :, 32], in0=asd[:],
                                    in1=padmaskh, op=mybir.AluOpType.add)
            nc.vector.tensor_tensor(out=tmp2[:], in0=st2[:, :, 0:32],
                                    in1=vd2bc.rearrange("p (o w) -> p o w", o=1).to_broadcast([P, NBLK, 32]),
                                    op=mybir.AluOpType.mult)
            nc.vector.tensor_reduce(out=asd[:], in_=tmp2[:],
                                    axis=mybir.AxisListType.X,
                                    op=mybir.AluOpType.add)
            nc.vector.tensor_copy(out=st2[:, :, 33], in_=asd[:])
            nc.vector.memset(st2[:, :, 34:ROWH], 0.0)
            nc.sync.dma_start(
                agin2[:].rearrange("(g p) w -> p g w", p=P), st2[:])
            # a_d2 per bucket via gather from agin2 col 33 (local; runs
            # while the AllGather is in flight)
            for b in range(NB):
                for (col0, cols, tp, t0) in meta_ad[b]:
                    dma_gather_raw(
                        nc.gpsimd,
                        adcol2[:, b, t0:t0 + tp].rearrange("p (g o) -> p g o", o=1),
                        agin2[:, 33:34], adixt[:, col0:col0 + cols],
                        tp * P, 1, ROWH, queue_num=nextq())
            nc.gpsimd.collective_compute(
                "AllGather", mybir.AluOpType.bypass,
                replica_groups=[list(range(NC))],
                ins=[agin2[:]], outs=[tab2[:]])

            f1pool.__exit__(None, None, None)

            # ---------------- layer 2
            edge_phase(1, tab2, L2W, lambda b: adcol2[:, b, :],
                       dt_row=DH, estep=ROWH)

            f2pool = tc.tile_pool(name="f2", bufs=1)
            spool = f2pool.__enter__()
            v2 = combine(1, L2W, spool)
            rec2 = spool.tile([P, NBLK], DT, tag="rec2")
            nc.vector.tensor_scalar_add(rec2[:], v2[:, :, 32], EPS)
            nc.vector.reciprocal(rec2[:], rec2[:])
            vst2 = spool.tile([P, NBLK, 32], DT, tag="vst2")
            nc.vector.tensor_tensor(out=vst2[:], in0=v2[:, :, 0:32],
                                    in1=rec2[:].to_broadcast([P, NBLK, 32]),
                                    op=mybir.AluOpType.mult)
            hf = spool.tile([P, NBLK, 32], DT, tag="hf")
            for u in range(0, NBLK, 4):
                nu = min(4, NBLK - u)
                tp2 = pspool.tile([32 * nu, P], DT, space="PSUM", tag="tps")
                nc.tensor.transpose(
                    out=tp2[:],
                    in_=vst2[:, u:u + nu, :].rearrange("p a b -> p (a b)"),
                    identity=ident[:])
                t2s = zpool.tile([32 * nu, P], DT, tag="t2s")
                nc.vector.tensor_copy(out=t2s[:], in_=tp2[:])
                hp2 = pspool.tile([P, nu * 32], DT, space="PSUM", tag="hps")
                nc.tensor.matmul(hp2[:], t2s[:], W2diag[0:32 * nu, 0:nu * 32],
                                 start=True, stop=True)
                nc.scalar.activation(
                    hf[:, u:u + nu, :],
                    hp2[:].rearrange("p (a b) -> p a b", a=nu),
                    mybir.ActivationFunctionType.Relu)
            # reuse vst2 as scratch for the Wl product (hf is final by now)
            nc.vector.tensor_tensor(out=vst2[:], in0=hf[:],
                                    in1=Wlbc.rearrange("p (o w) -> p o w", o=1).to_broadcast([P, NBLK, 32]),
                                    op=mybir.AluOpType.mult)
            ycol = spool.tile([P, NBLK], DT, tag="ycol")
            nc.vector.tensor_reduce(out=ycol[:], in_=vst2[:],
                                    axis=mybir.AxisListType.X,
                                    op=mybir.AluOpType.add)
            if bl != 0.0:
                nc.vector.tensor_scalar_add(ycol[:], ycol[:], bl)
            nc.sync.dma_start(
                y_d[:].rearrange("(g p) w -> p (g w)", p=P), ycol[:])
            f2pool.__exit__(None, None, None)

    nc.compile()
    return nc


def build_consts(weights):
    W1 = weights["W1"].astype(np.float32)
    W2 = weights["W2"].astype(np.float32)
    vs2 = (W2 @ weights["att_src2"]).astype(np.float32)
    vd2 = (W2 @ weights["att_dst2"]).astype(np.float32)
    Wl = weights["Wl"][:, 0].astype(np.float32)
    ct = np.zeros((P, 768), np.float32)
    for u in range(4):
        ct[3 * u:3 * u + 3, 0 + 32 * u:0 + 32 * u + 32] = W1
    for u in range(4):
        ct[32 * u:32 * u + 32, 128 + 32 * u:128 + 32 * u + 32] = W2
    ct[:, 256:288] = vs2[None, :]
    ct[:, 288:320] = vd2[None, :]
    ct[:, 320:352] = Wl[None, :]
    ct[:, 352:480] = np.eye(P, dtype=np.float32)
    pmh = np.zeros((P, NBLK), np.float32)
    pmh[84:128, NBLK - 1] = -30000.0
    ct[:, 640:640 + NBLK] = pmh
    return ct


def build_inputs(x, prep, weights):
    node2rank = prep["node2rank"]
    xr = np.zeros((NRANK, 3), np.float32)
    xr[node2rank] = x
    vs1 = (weights["W1"] @ weights["att_src1"]).astype(np.float32)
    vd1 = (weights["W1"] @ weights["att_dst1"]).astype(np.float32)
    a_s1 = xr @ vs1
    a_d1 = xr @ vd1
    pad = np.arange(NRANK).reshape(NC, NPC)[:, NPC_REAL:].ravel()
    a_s1[pad] = A_S_PAD
    tab1 = np.zeros((NRANK, ROWF), np.float32)
    tab1[:, 0:3] = xr
    tab1[:, 3] = a_s1
    ct = build_consts(weights)
    per_core = []
    for c in range(NC):
        adl = a_d1[c * NPC:(c + 1) * NPC]
        adc = np.zeros((P, NB, NBLK), np.float32)
        for b in range(NB):
            perm = prep["perms"][c, b]
            adc[:, b] = adl[perm].reshape(NBLK, P).T
        per_core.append({
            "tab1": tab1, "adc1": adc,
            "gidx": prep["gidx"][c], "cidx": prep["cidx"][c],
            "adidx": prep["adidx"][c], "consts": ct,
        })
    return per_core


_CACHE = {}
LAST_EXEC_NS = None
LAST_RESULTS = None


def kernel(**inputs):
    x = np.asarray(inputs["x"], np.float32)
    edge_index = np.asarray(inputs["edge_index"])
    weights = {k: np.asarray(v, np.float32) for k, v in inputs.items()
               if k not in ("x", "edge_index")}

    key = edge_index.tobytes()[:64]  # cheap cache key
    if key not in _CACHE:
        prep = preprocess(edge_index)
        nc = build_program(prep, weights)
        _CACHE[key] = (prep, nc)
    prep, nc = _CACHE[key]

    in_maps = build_inputs(x, prep, weights)
    import os
    trace = bool(int(os.environ.get("GAT_TRACE", "0")))
    res = run_bass_kernel_spmd(nc, in_maps, core_ids=list(range(NC)),
                               trace=trace)
    global LAST_EXEC_NS, LAST_RESULTS
    LAST_EXEC_NS = res.exec_time_ns
    LAST_RESULTS = res
    y = np.zeros((N, 1), np.float32)
    yr = np.concatenate([res.results[c]["y"] for c in range(NC)], axis=0)
    y[:, 0] = yr[prep["node2rank"], 0]
    return y


if __name__ == "__main__":
    d = np.load("/root/problem/work/inputs.npz")
    inp = {k: d[k] for k in d.files}
    y = kernel(**inp)
    y_ref = np.load("/root/problem/work/y_ref.npy")
    rel = np.abs(y - y_ref).max() / np.abs(y_ref).max()
    print("rel err:", rel)


# revision 10
# speedup vs baseline: 1.4749x; 1.4749x over previous
"""Trainium2 Bass kernel for a 2-layer GAT (GATConv x2 + linear head).

Strategy (8 NeuronCores, dst-node sharded, zero cross-core reduction):
  - Nodes are snake-dealt to 8 cores by in-degree (load balance); each core
    owns 12500 nodes (+44 pad ranks -> 12544 = 98 blocks of 128).
  - Global rank r = core*12544 + local. Node tables are indexed by rank.
  - Self-loops are excluded from the edge rectangles: their contribution is
    a per-node term (host-computed for layer 1, on-device for layer 2)
    added at finalize.
  - Non-self edges are grouped per (src-bucket b of 25088 ranks, dst-block
    of 128 nodes). Within each (b, block), dst nodes are ordered by their
    bucket-b in-degree so the slot rectangle [128 nodes x k slots] is
    near-tight.
  - Gather: custom SWDGE dma_gather with int16 bucket-relative indices and
    relaxed element size (layer1 row = 16B: x(3)+a_s; layer2 row = 66B fp16:
    x2(32)+a_s2), table rows strided 256B. Gathers are batched ~8k indices
    per instruction; the Q7 descriptor generation (~2.2ns/idx, serial on
    the Pool engine) is the primary bottleneck of this problem.
  - Per-edge softmax: z = a_s[src] + a_d[dst] (a_d is a per-partition column
    because dst == partition), Prelu+Exp on ACT, weight & segment-sum via an
    in-place multiply + strided free-dim tensor_reduce on DVE. The segment
    max subtraction is skipped (logits are in [-5, 5]; exp is safe and the
    softmax is shift-invariant).
  - Per-bucket partial sums accumulate in an SBUF tile [128, 98 blocks, W]
    (dst partition = node-within-block, in the bucket's degree-sorted perm
    order), stored once per bucket to a 256B-row DRAM table (store issued
    one sbatch late so it never heads the Sync queue in front of gidx slab
    loads), then combined across buckets with 4 small SWDGE gathers
    (perm -> rank order, issued as soon as each bucket's store lands) +
    DVE adds. No scatter-add, no DRAM zero-init.
  - W1/W2 are folded OUT of the tables (aggregation is linear in h): the
    tables carry raw features; W is applied once per layer at finalize via a
    PE transpose + block-diagonal-W matmul per 4 blocks.
  - Layer-2 node table is exchanged with a single AllGather (3.2MB/core).

kernel(**inputs) -> np.ndarray [100000, 1] float32.
"""

import numpy as np

import concourse.bass as bass
import concourse.mybir as mybir
import concourse.tile as tile
from concourse import bacc, ap_utils
from concourse._compat import exact_div
from concourse.bass_utils import run_bass_kernel_spmd

# ---------------------------------------------------------------- constants
N = 100000
E = 3200000
NC = 8
P = 128
NPC_REAL = 12500
NPC = 12544
NBLK = NPC // P            # 98
BUCKET = 2 * NPC           # 25088
NB = 4
NRANK = NC * NPC           # 100352
ROWF = 64                  # f32 table row stride in elems (256B)
ROWH = 128                 # fp16 table row stride in elems (256B)
L1W = 4                    # layer-1 gather width: x(3) + a_s1
L2W = 33                   # layer-2 gather width: x2(32) + a_s2
NEG = 0.2
A_S_PAD = -1.0e9
EPS = 1e-16
import os as _os
CAPS = int(_os.environ.get("GAT_CAPS", "63"))  # slots (x128 idxs) per gather
SBATCH = 4 * CAPS          # slots per compute batch
GSLAB = 4096               # gidx slab columns (int16) per load
HB = NBLK // 2             # finalize half size (49 blocks)
DT = mybir.dt.float32
DH = mybir.dt.float16
DI = mybir.dt.int16


# ------------------------------------------------------- raw SWDGE gather
def dma_gather_raw(gp, out_ap, in_ap, idxs_ap, num_idxs, elem_size, elem_step,
                   queue_num=0):
    assert idxs_ap.dtype == DI
    assert in_ap.dtype == out_ap.dtype
    assert in_ap.space == bass.MemorySpace.DRAM
    assert ap_utils.ap_is_contiguous(out_ap.ap[1:])
    assert ap_utils.ap_is_contiguous(idxs_ap.ap[1:])
    assert in_ap.ap[-1][1] == out_ap.ap[-1][1] == elem_size
    assert out_ap.ap[0][1] * out_ap.ap[1][1] >= num_idxs
    assert in_ap.ap[0][0] == elem_step
    stride_bytes_256 = exact_div(elem_step * mybir.dt.size(in_ap.dtype), 256)
    assert 0 < stride_bytes_256 < 256
    _in_ap = gp.lower_ap_dma(in_ap, for_custom_bir_dma=True)
    _idxs_ap = gp.lower_ap(idxs_ap)
    _out_ap = gp.lower_ap(out_ap)
    return gp.add_instruction(
        mybir.InstDMAGatherAnt(
            name=gp.bass.get_next_instruction_name(),
            ins=[*_in_ap, _idxs_ap, gp.lower_val_access(gp.to_reg(num_idxs))],
            outs=[_out_ap],
            transpose=False,
            num_idxs=num_idxs,
            elem_size=elem_size,
            stride_bytes_256=stride_bytes_256,
            gen_mode=0,
            # single_packet coalesces each engine's descs into one packet;
            # packets cap at 64 descs / 4KB, i.e. 1024 idxs
            single_packet=num_idxs <= 1024,
            queue_num=queue_num,
            sbuf_tokens_per_rank=0,
            sbuf_free_dim_per_rank=0,
            sbuf_free_dim_pad_per_rank=0,
            sbuf_byte_offset=0,
        ))


def wrap16(idx):
    """[n] int -> SWDGE wrapped idx layout [128, n/16] int16 (8x replicated)."""
    n = len(idx)
    n16 = ((n + 15) // 16) * 16
    buf = np.full(n16, -1, np.int16)
    buf[:n] = idx
    w = buf.reshape(n16 // 16, 16).T
    return np.tile(w, (8, 1))


# ------------------------------------------------------- host preprocessing
def preprocess(edge_index):
    # self-loops are handled by a dedicated per-node term; only the raw
    # edges (including any natural src==dst duplicates) go in rectangles
    src = edge_index[0].astype(np.int64)
    dst = edge_index[1].astype(np.int64)

    deg = np.bincount(dst, minlength=N)
    order = np.argsort(-deg, kind="stable")
    pos = np.arange(N)
    rnd, lane = pos // NC, pos % NC
    core = np.where(rnd % 2 == 0, lane, NC - 1 - lane)
    node2rank = np.empty(N, np.int64)
    node2rank[order] = core * NPC + rnd

    srank = node2rank[src]
    drank = node2rank[dst]
    dcore = drank // NPC

    per_core = []
    counts = np.zeros((NC, NB, NPC), np.int64)
    for c in range(NC):
        m = dcore == c
        s_c, d_c = srank[m], drank[m] % NPC
        b_c = s_c // BUCKET
        per_core.append((s_c, d_c, b_c))
        for b in range(NB):
            mm = b_c == b
            counts[c, b] = np.bincount(d_c[mm], minlength=NPC)

    perms = np.empty((NC, NB, NPC), np.int64)
    for c in range(NC):
        for b in range(NB):
            perms[c, b] = np.argsort(-counts[c, b], kind="stable")

    # unified k per (bucket, block) across cores
    kk = np.zeros((NB, NBLK), np.int64)
    for b in range(NB):
        cnt = np.take_along_axis(counts[:, b], perms[:, b], axis=1)
        kk[b] = cnt.reshape(NC, NBLK, P).max(axis=(0, 2))

    # compute groups: consecutive same-k blocks, m*k <= SBATCH
    groups = []  # (b, g0, m, k)
    for b in range(NB):
        g = 0
        while g < NBLK:
            k = int(kk[b, g])
            if k == 0:
                g += 1
                continue
            mlim = max(1, SBATCH // k)
            m = 1
            while (m < mlim and g + m < NBLK and kk[b, g + m] == k):
                m += 1
            groups.append((b, g, m, k))
            g += m

    # sbatches: consecutive same-bucket groups, total slots <= SBATCH
    sbatches = []
    gi = 0
    while gi < len(groups):
        b = groups[gi][0]
        glist = []
        S = 0
        while gi < len(groups) and groups[gi][0] == b:
            _, g0, m, k = groups[gi]
            if S + m * k > SBATCH:
                break
            glist.append((g0, m, k, S))
            S += m * k
            gi += 1
        sbatches.append(dict(b=b, glist=glist, S=S))

    # per-core gather index streams, in sbatch/piece order
    gstream = [[] for _ in range(NC)]
    col = 0
    for sb in sbatches:
        b = sb["b"]
        rects = []
        for c in range(NC):
            s_c, d_c, b_c = per_core[c]
            mm = b_c == b
            sb_s, sb_d = s_c[mm], d_c[mm]
            o = np.argsort(sb_d, kind="stable")
            sb_s, sb_d = sb_s[o], sb_d[o]
            starts = np.searchsorted(sb_d, np.arange(NPC))
            ends = np.searchsorted(sb_d, np.arange(NPC) + 1)
            rect = np.full((sb["S"], P), NPC_REAL, np.int64)  # dummy row
            for (g0, m, k, off) in sb["glist"]:
                nodes = perms[c, b, g0 * P:(g0 + m) * P]
                for u in range(m):
                    nd = nodes[u * P:(u + 1) * P]
                    for p, nloc in enumerate(nd):
                        s0, s1 = starts[nloc], ends[nloc]
                        cnt = s1 - s0
                        row0 = off + u * k
                        vals = np.sort(sb_s[s0:s1] - BUCKET * b)
                        rect[row0:row0 + cnt, p] = vals
            rects.append(rect)
        pieces = []
        t0 = 0
        while t0 < sb["S"]:
            tp = min(CAPS, sb["S"] - t0)
            pieces.append((col, tp * 8, tp, t0))
            for c in range(NC):
                part = rects[c][t0:t0 + tp, :].reshape(-1)
                gstream[c].append(wrap16(part))
            col += tp * 8
            t0 += tp
        sb["pieces"] = pieces
    gidx_arr = [np.concatenate(gstream[c], axis=1) for c in range(NC)]

    # combine gather idxs: for final (p, g) the position of node g*128+p in
    # the bucket-b partial table (row = p_b*NBLK + g_b).
    cstream = [[] for _ in range(NC)]
    meta_c = []   # per bucket: list of (col0, cols, tp, t0)
    ccol = 0
    for b in range(NB):
        pieces = []
        t0 = 0
        while t0 < NBLK:
            tp = min(CAPS, NBLK - t0)
            pieces.append((ccol, tp * 8, tp, t0))
            ccol += tp * 8
            t0 += tp
        meta_c.append(pieces)
    for c in range(NC):
        for b in range(NB):
            inv = np.empty(NPC, np.int64)
            inv[perms[c, b]] = np.arange(NPC)
            pos = (inv % P) * NBLK + (inv // P)
            for (col0, cols, tp, t0) in meta_c[b]:
                part = pos[t0 * P:(t0 + tp) * P]
                cstream[c].append(wrap16(part))
    cidx_arr = [np.concatenate(cstream[c], axis=1) for c in range(NC)]

    # a_d idx stream: per bucket, perm order (local ranks), pieces of CAPS
    adstream = [[] for _ in range(NC)]
    meta_ad = []
    acol = 0
    for b in range(NB):
        pieces = []
        t0 = 0
        while t0 < NBLK:
            tp = min(CAPS, NBLK - t0)
            pieces.append((acol, tp * 8, tp, t0))
            for c in range(NC):
                part = perms[c, b][t0 * P:(t0 + tp) * P]
                adstream[c].append(wrap16(part))
            acol += tp * 8
            t0 += tp
        meta_ad.append(pieces)
    adidx_arr = [np.concatenate(adstream[c], axis=1) for c in range(NC)]

    return dict(node2rank=node2rank, sbatches=sbatches, meta_c=meta_c,
                meta_ad=meta_ad, gidx=gidx_arr, cidx=cidx_arr,
                adidx=adidx_arr, perms=perms, gcols=col, ccols=ccol,
                adcols=acol)


# ------------------------------------------------------- program builder
def build_program(prep, weights):
    sbatches = prep["sbatches"]
    meta_c, meta_ad = prep["meta_c"], prep["meta_ad"]
    b1 = weights["b1"]; b2 = weights["b2"]
    bl = float(weights["bl"][0])
    if np.abs(b1).max() > 0 or np.abs(b2).max() > 0:
        raise NotImplementedError("nonzero b1/b2")

    nc = bacc.Bacc("TRN2", target_bir_lowering=False, debug=False,
                   enable_asserts=False, num_devices=NC,
                   num_swdge_queues=4,
                   dynamic_dma_scratch_size=32768)

    # ---- external tensors
    adc1 = nc.dram_tensor("adc1", [P, NB, NBLK], DT, kind="ExternalInput")
    selfc1_d = nc.dram_tensor("selfc1", [P, NBLK, L1W], DT, kind="ExternalInput")
    gidx_d = nc.dram_tensor("gidx", [P, prep["gcols"]], DI, kind="ExternalInput")
    cidx_d = nc.dram_tensor("cidx", [P, prep["ccols"]], DI, kind="ExternalInput")
    adidx_d = nc.dram_tensor("adidx", [P, prep["adcols"]], DI, kind="ExternalInput")
    consts = nc.dram_tensor("consts", [P, 768], DT, kind="ExternalInput")
    y_d = nc.dram_tensor("y", [NPC, 1], DT, kind="ExternalOutput")

    # ---- internal DRAM
    tab1 = nc.dram_tensor("tab1", [NRANK, ROWF], DT, kind="ExternalInput")
    agin2 = nc.dram_tensor("agin2", [NPC, ROWH], DH)
    tab2 = nc.dram_tensor("tab2", [NRANK, ROWH], DH, addr_space="Shared")
    # per-bucket partial tables, 256B rows, row index = p*NBLK + g
    part_t = [[nc.dram_tensor(f"part{li}_{b}", [NPC, ROWH], DH)
               for b in range(NB)] for li in range(2)]

    with tile.TileContext(nc) as tc:
        with tc.tile_pool(name="const", bufs=1) as cpool, \
             tc.tile_pool(name="chunk", bufs=2) as chpool, \
             tc.tile_pool(name="small", bufs=3) as zpool, \
             tc.tile_pool(name="gix", bufs=2) as gixpool, \
             tc.tile_pool(name="pb", bufs=2) as pbpool, \
             tc.tile_pool(name="cg", bufs=4) as cgpool, \
             tc.tile_pool(name="psum", bufs=2, space="PSUM") as pspool:

            ct = cpool.tile([P, 768], DT)
            nc.sync.dma_start(ct[:], consts[:])
            W1diag = ct[:, 0:128]      # valid on partitions 0:12
            W2diag = ct[:, 128:256]
            vs2bc = ct[:, 256:288]
            vd2bc = ct[:, 288:320]
            Wlbc = ct[:, 320:352]
            ident = ct[:, 352:480]
            padmaskh = ct[:, 640:640 + NBLK]

            adcol1 = cpool.tile([P, NB, NBLK], DT, tag="adcol1")
            nc.sync.dma_start(adcol1[:], adc1[:])
            adcol2 = cpool.tile([P, NB, NBLK], DH, tag="adcol2")
            selfc1 = cpool.tile([P, NBLK, L1W], DT, tag="selfc1")
            nc.sync.dma_start(selfc1[:], selfc1_d[:])
            selft2 = cpool.tile([P, NBLK, L2W], DH, tag="selft2")
            cixt = cpool.tile([P, prep["ccols"]], DI, tag="cixt")
            nc.sync.dma_start(cixt[:], cidx_d[:])
            adixt = cpool.tile([P, prep["adcols"]], DI, tag="adixt")
            nc.sync.dma_start(adixt[:], adidx_d[:])

            qrr = [0]

            def nextq():
                qrr[0] = (qrr[0] + 1) % 4
                return qrr[0]

            def cgather(li, b, W):
                cg = cgpool.tile([P, NBLK, W], DH, tag="cg")
                for (col0, cols, tp, t0) in meta_c[b]:
                    dma_gather_raw(
                        nc.gpsimd, cg[:, t0:t0 + tp, :],
                        part_t[li][b][:, 0:W],
                        cixt[:, col0:col0 + cols], tp * P, W, ROWH,
                        queue_num=nextq())
                return cg

            def edge_phase(li, tab, W, adcol_fn, dt_row=DT, estep=ROWF,
                           slab0=None):
                slab = {"tile": slab0, "base": 0 if slab0 is not None else -1}

                def gix(col0, cols):
                    if (slab["tile"] is None or col0 < slab["base"]
                            or col0 + cols > slab["base"] + GSLAB):
                        t = gixpool.tile([P, GSLAB], DI, tag="gslab")
                        base = col0
                        csz = min(GSLAB, prep["gcols"] - base)
                        nc.sync.dma_start(t[:, 0:csz], gidx_d[:, base:base + csz])
                        slab["tile"], slab["base"] = t, base
                    b0 = col0 - slab["base"]
                    return slab["tile"][:, b0:b0 + cols]

                cgs = {}
                pb = None
                prev_b = -1
                pend = None   # (pb_tile, bucket) awaiting store

                def flush(pend_):
                    pb_, b_ = pend_
                    dest = part_t[li][b_][:].rearrange(
                        "(p g) w -> p g w", p=P)[:, :, 0:W]
                    nc.sync.dma_start(dest, pb_[:])

                for sb in sbatches:
                    b = sb["b"]
                    if b != prev_b:
                        if pb is not None:
                            assert pend is None
                            pend = (pb, prev_b)
                        pb = pbpool.tile([P, NBLK, W], DH, tag="pb")
                        nc.vector.memset(pb[:], 0.0)
                        prev_b = b
                    S = sb["S"]
                    chunk = chpool.tile([P, SBATCH, W], dt_row, tag="chunk")
                    for (col0, cols, tp, t0) in sb["pieces"]:
                        dma_gather_raw(
                            nc.gpsimd, chunk[:, t0:t0 + tp, :],
                            tab[BUCKET * b:BUCKET * (b + 1), 0:W],
                            gix(col0, cols), tp * P, W, estep,
                            queue_num=nextq())
                    if pend is not None:
                        # software pipeline: the store of the previous
                        # bucket's partials goes out after this sbatch's
                        # gathers; its combine gather follows
                        flush(pend)
                        cgs[pend[1]] = cgather(li, pend[1], W)
                        pend = None
                    z = zpool.tile([P, SBATCH], DT, tag="z")
                    ad = adcol_fn(b)
                    for (g0, m, k, off) in sb["glist"]:
                        nc.vector.tensor_tensor(
                            out=z[:, off:off + m * k].rearrange(
                                "p (m k) -> p m k", m=m),
                            in0=chunk[:, off:off + m * k, W - 1].rearrange(
                                "p (m k) -> p m k", m=m),
                            in1=ad[:, g0:g0 + m].rearrange(
                                "p (m o) -> p m o", o=1).to_broadcast([P, m, k]),
                            op=mybir.AluOpType.add)
                    zp = zpool.tile([P, SBATCH], DT, tag="zp")
                    nc.scalar.activation(zp[:, 0:S], z[:, 0:S],
                                         mybir.ActivationFunctionType.Prelu,
                                         alpha=NEG)
                    ex = zpool.tile([P, SBATCH], DT, tag="ex")
                    nc.scalar.activation(ex[:, 0:S], zp[:, 0:S],
                                         mybir.ActivationFunctionType.Exp)
                    nc.vector.tensor_tensor(
                        out=chunk[:, 0:S, 0:W - 1],
                        in0=chunk[:, 0:S, 0:W - 1],
                        in1=ex[:, 0:S].to_broadcast([P, S, W - 1]),
                        op=mybir.AluOpType.mult)
                    with nc.allow_low_precision(
                            reason="fp16 partials; DVE accumulates f32 "
                                   "internally, rel tol 2e-2"):
                        for (g0, m, k, off) in sb["glist"]:
                            nc.vector.tensor_reduce(
                                out=pb[:, g0:g0 + m, 0:W - 1],
                                in_=chunk[:, off:off + m * k, 0:W - 1].rearrange(
                                    "p (m k) w -> p m w k", m=m),
                                axis=mybir.AxisListType.X,
                                op=mybir.AluOpType.add)
                            nc.vector.tensor_reduce(
                                out=pb[:, g0:g0 + m, W - 1],
                                in_=ex[:, off:off + m * k].rearrange(
                                    "p (m k) -> p m k", m=m),
                                axis=mybir.AxisListType.X,
                                op=mybir.AluOpType.add)
                flush((pb, prev_b))
                cgs[prev_b] = cgather(li, prev_b, W)
                return cgs

            def combine(cgs, W, selftile, spool, tag):
                v = spool.tile([P, NBLK, W], DT, tag=tag)
                nc.vector.tensor_tensor(out=v[:], in0=cgs[0][:], in1=cgs[1][:],
                                        op=mybir.AluOpType.add)
                nc.vector.tensor_tensor(out=v[:], in0=v[:], in1=cgs[2][:],
                                        op=mybir.AluOpType.add)
                nc.vector.tensor_tensor(out=v[:], in0=v[:], in1=cgs[3][:],
                                        op=mybir.AluOpType.add)
                # self-loop term: [values 0:W-1, den at W-1]
                nc.vector.tensor_tensor(out=v[:, :, 0:W], in0=v[:, :, 0:W],
                                        in1=selftile[:],
                                        op=mybir.AluOpType.add)
                return v

            # ---------------- layer 1
            cgs1 = edge_phase(0, tab1, L1W, lambda b: adcol1[:, b, :])

            f1pool = tc.tile_pool(name="f1", bufs=1)
            spool = f1pool.__enter__()
            v1 = combine(cgs1, L1W, selfc1, spool, "v1")
            rec1 = spool.tile([P, NBLK], DT, tag="rec1")
            nc.vector.tensor_scalar_add(rec1[:], v1[:, :, L1W - 1], EPS)
            nc.vector.reciprocal(rec1[:], rec1[:])
            vst1 = spool.tile([P, NBLK, 3], DT, tag="vst1")
            nc.vector.tensor_tensor(out=vst1[:], in0=v1[:, :, 0:3],
                                    in1=rec1[:].to_broadcast([P, NBLK, 3]),
                                    op=mybir.AluOpType.mult)
            # finalize-1 in halves of 49 blocks: W1 sandwich, a_s2/a_d2,
            # self-term, agin2 store per half (overlaps compute / DMA)
            for h in range(2):
                u0, u1 = h * HB, (h + 1) * HB
                sth = spool.tile([P, HB, ROWH], DH, tag="sth")
                for u in range(u0, u1, 4):
                    nu = min(4, u1 - u)
                    tp1 = pspool.tile([3 * nu, P], DT, space="PSUM", tag="tps")
                    nc.tensor.transpose(
                        out=tp1[:],
                        in_=vst1[:, u:u + nu, :].rearrange("p a b -> p (a b)"),
                        identity=ident[:])
                    t1s = zpool.tile([3 * nu, P], DT, tag="t1s")
                    nc.vector.tensor_copy(out=t1s[:], in_=tp1[:])
                    hp = pspool.tile([P, nu * 32], DT, space="PSUM", tag="hps")
                    nc.tensor.matmul(hp[:], t1s[:],
                                     W1diag[0:3 * nu, 0:nu * 32],
                                     start=True, stop=True)
                    nc.scalar.activation(
                        sth[:, u - u0:u - u0 + nu, 0:32],
                        hp[:].rearrange("p (a b) -> p a b", a=nu),
                        mybir.ActivationFunctionType.Relu)
                tmp2 = spool.tile([P, HB, 32], DT, tag="tmp2")
                asd = spool.tile([P, HB], DT, tag="asd")
                # a_s2 (+ pad mask)
                nc.vector.tensor_tensor(
                    out=tmp2[:], in0=sth[:, :, 0:32],
                    in1=vs2bc.rearrange("p (o w) -> p o w", o=1).to_broadcast(
                        [P, HB, 32]),
                    op=mybir.AluOpType.mult)
                nc.vector.tensor_reduce(out=asd[:], in_=tmp2[:],
                                        axis=mybir.AxisListType.X,
                                        op=mybir.AluOpType.add)
                nc.vector.tensor_tensor(out=sth[:, :, 32], in0=asd[:],
                                        in1=padmaskh[:, u0:u1],
                                        op=mybir.AluOpType.add)
                # a_d2
                nc.vector.tensor_tensor(
                    out=tmp2[:], in0=sth[:, :, 0:32],
                    in1=vd2bc.rearrange("p (o w) -> p o w", o=1).to_broadcast(
                        [P, HB, 32]),
                    op=mybir.AluOpType.mult)
                nc.vector.tensor_reduce(out=asd[:], in_=tmp2[:],
                                        axis=mybir.AxisListType.X,
                                        op=mybir.AluOpType.add)
                nc.vector.tensor_copy(out=sth[:, :, 33], in_=asd[:])
                nc.vector.memset(sth[:, :, 34:ROWH], 0.0)
                # layer-2 self term: es2 = exp(prelu(a_s2+pad + a_d2))
                zs = zpool.tile([P, HB], DT, tag="zs")
                nc.vector.tensor_tensor(out=zs[:], in0=sth[:, :, 32],
                                        in1=asd[:], op=mybir.AluOpType.add)
                nc.scalar.activation(zs[:], zs[:],
                                     mybir.ActivationFunctionType.Prelu,
                                     alpha=NEG)
                nc.scalar.activation(zs[:], zs[:],
                                     mybir.ActivationFunctionType.Exp)
                nc.vector.tensor_tensor(
                    out=selft2[:, u0:u1, 0:32], in0=sth[:, :, 0:32],
                    in1=zs[:].to_broadcast([P, HB, 32]),
                    op=mybir.AluOpType.mult)
                nc.vector.tensor_copy(out=selft2[:, u0:u1, 32], in_=zs[:])
                nc.scalar.dma_start(
                    agin2[:].rearrange("(g p) w -> p g w", p=P)[:, u0:u1, :],
                    sth[:])
            # a_d2 per bucket via gather from agin2 col 33 (local; runs
            # while peers converge on the AllGather)
            for b in range(NB):
                for (col0, cols, tp, t0) in meta_ad[b]:
                    dma_gather_raw(
                        nc.gpsimd,
                        adcol2[:, b, t0:t0 + tp].rearrange("p (g o) -> p g o", o=1),
                        agin2[:, 33:34], adixt[:, col0:col0 + cols],
                        tp * P, 1, ROWH, queue_num=nextq())
            nc.gpsimd.collective_compute(
                "AllGather", mybir.AluOpType.bypass,
                replica_groups=[list(range(NC))],
                ins=[agin2[:]], outs=[tab2[:]])
            # preload layer-2's first gidx slab while the AllGather flies
            slab2 = gixpool.tile([P, GSLAB], DI, tag="gslab")
            nc.sync.dma_start(slab2[:], gidx_d[:, 0:GSLAB])

            f1pool.__exit__(None, None, None)

            # ---------------- layer 2
            cgs2 = edge_phase(1, tab2, L2W, lambda b: adcol2[:, b, :],
                              dt_row=DH, estep=ROWH, slab0=slab2)

            f2pool = tc.tile_pool(name="f2", bufs=1)
            spool = f2pool.__enter__()
            v2 = combine(cgs2, L2W, selft2, spool, "v2")
            rec2 = spool.tile([P, NBLK], DT, tag="rec2")
            nc.vector.tensor_scalar_add(rec2[:], v2[:, :, 32], EPS)
            nc.vector.reciprocal(rec2[:], rec2[:])
            vst2 = spool.tile([P, NBLK, 32], DT, tag="vst2")
            nc.vector.tensor_tensor(out=vst2[:], in0=v2[:, :, 0:32],
                                    in1=rec2[:].to_broadcast([P, NBLK, 32]),
                                    op=mybir.AluOpType.mult)
            hf = spool.tile([P, NBLK, 32], DT, tag="hf")
            for u in range(0, NBLK, 4):
                nu = min(4, NBLK - u)
                tp2 = pspool.tile([32 * nu, P], DT, space="PSUM", tag="tps")
                nc.tensor.transpose(
                    out=tp2[:],
                    in_=vst2[:, u:u + nu, :].rearrange("p a b -> p (a b)"),
                    identity=ident[:])
                t2s = zpool.tile([32 * nu, P], DT, tag="t2s")
                nc.vector.tensor_copy(out=t2s[:], in_=tp2[:])
                hp2 = pspool.tile([P, nu * 32], DT, space="PSUM", tag="hps")
                nc.tensor.matmul(hp2[:], t2s[:], W2diag[0:32 * nu, 0:nu * 32],
                                 start=True, stop=True)
                nc.scalar.activation(
                    hf[:, u:u + nu, :],
                    hp2[:].rearrange("p (a b) -> p a b", a=nu),
                    mybir.ActivationFunctionType.Relu)
            # reuse vst2 as scratch for the Wl product (hf is final by now)
            nc.vector.tensor_tensor(out=vst2[:], in0=hf[:],
                                    in1=Wlbc.rearrange("p (o w) -> p o w", o=1).to_broadcast([P, NBLK, 32]),
                                    op=mybir.AluOpType.mult)
            ycol = spool.tile([P, NBLK], DT, tag="ycol")
            nc.vector.tensor_reduce(out=ycol[:], in_=vst2[:],
                                    axis=mybir.AxisListType.X,
                                    op=mybir.AluOpType.add)
            if bl != 0.0:
                nc.vector.tensor_scalar_add(ycol[:], ycol[:], bl)
            nc.scalar.dma_start(
                y_d[:].rearrange("(g p) w -> p (g w)", p=P), ycol[:])
            f2pool.__exit__(None, None, None)

    nc.compile()
    return nc


def build_consts(weights):
    W1 = weights["W1"].astype(np.float32)
    W2 = weights["W2"].astype(np.float32)
    vs2 = (W2 @ weights["att_src2"]).astype(np.float32)
    vd2 = (W2 @ weights["att_dst2"]).astype(np.float32)
    Wl = weights["Wl"][:, 0].astype(np.float32)
    ct = np.zeros((P, 768), np.float32)
    for u in range(4):
        ct[3 * u:3 * u + 3, 0 + 32 * u:0 + 32 * u + 32] = W1
    for u in range(4):
        ct[32 * u:32 * u + 32, 128 + 32 * u:128 + 32 * u + 32] = W2
    ct[:, 256:288] = vs2[None, :]
    ct[:, 288:320] = vd2[None, :]
    ct[:, 320:352] = Wl[None, :]
    ct[:, 352:480] = np.eye(P, dtype=np.float32)
    pmh = np.zeros((P, NBLK), np.float32)
    pmh[84:128, NBLK - 1] = -30000.0
    ct[:, 640:640 + NBLK] = pmh
    return ct


def build_inputs(x, prep, weights):
    node2rank = prep["node2rank"]
    xr = np.zeros((NRANK, 3), np.float32)
    xr[node2rank] = x
    vs1 = (weights["W1"] @ weights["att_src1"]).astype(np.float32)
    vd1 = (weights["W1"] @ weights["att_dst1"]).astype(np.float32)
    a_s1 = xr @ vs1
    a_d1 = xr @ vd1
    pad = np.arange(NRANK).reshape(NC, NPC)[:, NPC_REAL:].ravel()
    a_s1[pad] = A_S_PAD
    tab1 = np.zeros((NRANK, ROWF), np.float32)
    tab1[:, 0:3] = xr
    tab1[:, 3] = a_s1
    # layer-1 self-loop term per node (rank order): exp(prelu(as+ad))*[x, 1]
    zs = a_s1 + a_d1
    es = np.exp(np.where(zs >= 0, zs, NEG * zs)).astype(np.float32)
    es[pad] = 0.0
    selfc = np.concatenate([xr * es[:, None], es[:, None]], axis=1)
    ct = build_consts(weights)
    per_core = []
    for c in range(NC):
        adl = a_d1[c * NPC:(c + 1) * NPC]
        adc = np.zeros((P, NB, NBLK), np.float32)
        for b in range(NB):
            perm = prep["perms"][c, b]
            adc[:, b] = adl[perm].reshape(NBLK, P).T
        sc = selfc[c * NPC:(c + 1) * NPC].reshape(NBLK, P, L1W).transpose(1, 0, 2)
        per_core.append({
            "tab1": tab1, "adc1": adc, "selfc1": np.ascontiguousarray(sc),
            "gidx": prep["gidx"][c], "cidx": prep["cidx"][c],
            "adidx": prep["adidx"][c], "consts": ct,
        })
    return per_core


_CACHE = {}
LAST_EXEC_NS = None
LAST_RESULTS = None


def kernel(**inputs):
    x = np.asarray(inputs["x"], np.float32)
    edge_index = np.asarray(inputs["edge_index"])
    weights = {k: np.asarray(v, np.float32) for k, v in inputs.items()
               if k not in ("x", "edge_index")}

    key = edge_index.tobytes()[:64]  # cheap cache key
    if key not in _CACHE:
        prep = preprocess(edge_index)
        nc = build_program(prep, weights)
        _CACHE[key] = (prep, nc)
    prep, nc = _CACHE[key]

    in_maps = build_inputs(x, prep, weights)
    import os
    trace = bool(int(os.environ.get("GAT_TRACE", "0")))
    res = run_bass_kernel_spmd(nc, in_maps, core_ids=list(range(NC)),
                               trace=trace)
    global LAST_EXEC_NS, LAST_RESULTS
    LAST_EXEC_NS = res.exec_time_ns
    LAST_RESULTS = res
    y = np.zeros((N, 1), np.float32)
    yr = np.concatenate([res.results[c]["y"] for c in range(NC)], axis=0)
    y[:, 0] = yr[prep["node2rank"], 0]
    return y


if __name__ == "__main__":
    d = np.load("/root/problem/work/inputs.npz")
    inp = {k: d[k] for k in d.files}
    y = kernel(**inp)
    y_ref = np.load("/root/problem/work/y_ref.npy")
    rel = np.abs(y - y_ref).max() / np.abs(y_ref).max()
    print("rel err:", rel)


# revision 12
# speedup vs baseline: 1.6213x; 1.0992x over previous
"""Trainium2 Bass kernel for a 2-layer GAT (GATConv x2 + linear head).

Strategy (8 NeuronCores, dst-node sharded, zero cross-core reduction):
  - Nodes are snake-dealt to 8 cores by in-degree (load balance); each core
    owns 12500 nodes (+44 pad ranks -> 12544 = 98 blocks of 128).
  - Global rank r = core*12544 + local. Node tables are indexed by rank.
  - Self-loops are excluded from the edge rectangles: their contribution is
    a per-node term (host-computed for layer 1, on-device for layer 2)
    added at finalize.
  - Non-self edges are grouped per (src-bucket b of 25088 ranks, dst-block
    of 128 nodes). Within each (b, block), dst nodes are ordered by their
    bucket-b in-degree so the slot rectangle [128 nodes x k slots] is
    near-tight.
  - Gather: custom SWDGE dma_gather with int16 bucket-relative indices and
    relaxed element size (layer1 row = 16B: x(3)+a_s; layer2 row = 66B fp16:
    x2(32)+a_s2), table rows strided 256B. Gathers are batched ~8k indices
    per instruction; the Q7 descriptor generation (~2.2ns/idx, serial on
    the Pool engine) is the primary bottleneck of this problem.
  - Per-edge softmax: z = a_s[src] + a_d[dst] (a_d is a per-partition column
    because dst == partition), Prelu+Exp on ACT, weight & segment-sum via an
    in-place multiply + strided free-dim tensor_reduce on DVE. The segment
    max subtraction is skipped (logits are in [-5, 5]; exp is safe and the
    softmax is shift-invariant).
  - Per-bucket partial sums accumulate in an SBUF tile [128, 98 blocks, W]
    (dst partition = node-within-block, in the bucket's degree-sorted perm
    order), stored once per bucket to a 256B-row DRAM table (store issued
    one sbatch late so it never heads the Sync queue in front of gidx slab
    loads), then combined across buckets with 4 small SWDGE gathers
    (perm -> rank order, issued as soon as each bucket's store lands) +
    DVE adds. No scatter-add, no DRAM zero-init.
  - W1/W2 are folded OUT of the tables (aggregation is linear in h): the
    tables carry raw features; W is applied once per layer at finalize via a
    PE transpose + block-diagonal-W matmul per 4 blocks.
  - Layer-2 node table is exchanged with a single AllGather (3.2MB/core).

kernel(**inputs) -> np.ndarray [100000, 1] float32.
"""

import numpy as np

import concourse.bass as bass
import concourse.mybir as mybir
import concourse.tile as tile
from concourse import bacc, ap_utils
from concourse._compat import exact_div
from concourse.bass_utils import run_bass_kernel_spmd

# ---------------------------------------------------------------- constants
N = 100000
E = 3200000
NC = 8
P = 128
NPC_REAL = 12500
NPC = 12544
NBLK = NPC // P            # 98
BUCKET = 2 * NPC           # 25088
NB = 4
NRANK = NC * NPC           # 100352
ROWF = 64                  # f32 table row stride in elems (256B)
ROWH = 128                 # fp16 table row stride in elems (256B)
L1W = 4                    # layer-1 gather width: x(3) + a_s1
L2W = 33                   # layer-2 gather width: x2(32) + a_s2
NEG = 0.2
A_S_PAD = -1.0e9
EPS = 1e-16
import os as _os
CAPS = int(_os.environ.get("GAT_CAPS", "8"))   # slots (x128 idxs) per gather
SBATCH = 252               # slots per compute batch
GSLAB = 4096               # gidx slab columns (int16) per load
HB = NBLK // 2             # finalize half size (49 blocks)
DT = mybir.dt.float32
DH = mybir.dt.float16
DI = mybir.dt.int16


# ------------------------------------------------------- raw SWDGE gather
def dma_gather_raw(gp, out_ap, in_ap, idxs_ap, num_idxs, elem_size, elem_step,
                   queue_num=0):
    assert idxs_ap.dtype == DI
    assert in_ap.dtype == out_ap.dtype
    assert in_ap.space == bass.MemorySpace.DRAM
    assert ap_utils.ap_is_contiguous(out_ap.ap[1:])
    assert ap_utils.ap_is_contiguous(idxs_ap.ap[1:])
    assert in_ap.ap[-1][1] == out_ap.ap[-1][1] == elem_size
    assert out_ap.ap[0][1] * out_ap.ap[1][1] >= num_idxs
    assert in_ap.ap[0][0] == elem_step
    stride_bytes_256 = exact_div(elem_step * mybir.dt.size(in_ap.dtype), 256)
    assert 0 < stride_bytes_256 < 256
    _in_ap = gp.lower_ap_dma(in_ap, for_custom_bir_dma=True)
    _idxs_ap = gp.lower_ap(idxs_ap)
    _out_ap = gp.lower_ap(out_ap)
    return gp.add_instruction(
        mybir.InstDMAGatherAnt(
            name=gp.bass.get_next_instruction_name(),
            ins=[*_in_ap, _idxs_ap, gp.lower_val_access(gp.to_reg(num_idxs))],
            outs=[_out_ap],
            transpose=False,
            num_idxs=num_idxs,
            elem_size=elem_size,
            stride_bytes_256=stride_bytes_256,
            gen_mode=0,
            # single_packet coalesces each engine's descs into one packet;
            # packets cap at 64 descs / 4KB, i.e. 1024 idxs
            single_packet=num_idxs <= 1024,
            queue_num=queue_num,
            sbuf_tokens_per_rank=0,
            sbuf_free_dim_per_rank=0,
            sbuf_free_dim_pad_per_rank=0,
            sbuf_byte_offset=0,
        ))


def wrap16(idx):
    """[n] int -> SWDGE wrapped idx layout [128, n/16] int16 (8x replicated)."""
    n = len(idx)
    n16 = ((n + 15) // 16) * 16
    buf = np.full(n16, -1, np.int16)
    buf[:n] = idx
    w = buf.reshape(n16 // 16, 16).T
    return np.tile(w, (8, 1))


# ------------------------------------------------------- host preprocessing
def preprocess(edge_index):
    # self-loops are handled by a dedicated per-node term; only the raw
    # edges (including any natural src==dst duplicates) go in rectangles
    src = edge_index[0].astype(np.int64)
    dst = edge_index[1].astype(np.int64)

    deg = np.bincount(dst, minlength=N)
    order = np.argsort(-deg, kind="stable")
    pos = np.arange(N)
    rnd, lane = pos // NC, pos % NC
    core = np.where(rnd % 2 == 0, lane, NC - 1 - lane)
    node2rank = np.empty(N, np.int64)
    node2rank[order] = core * NPC + rnd

    def shard(n2r):
        srank = n2r[src]
        drank = n2r[dst]
        dcore = drank // NPC
        per_core = []
        counts = np.zeros((NC, NB, NPC), np.int64)
        for c in range(NC):
            m = dcore == c
            s_c, d_c = srank[m], drank[m] % NPC
            b_c = s_c // BUCKET
            per_core.append((s_c, d_c, b_c))
            for b in range(NB):
                mm = b_c == b
                counts[c, b] = np.bincount(d_c[mm], minlength=NPC)
        return per_core, counts

    per_core, counts = shard(node2rank)
    # reorder each core's locals by bucket-0 in-degree so perm[c, 0] becomes
    # the identity: bucket membership (= pair of cores) is invariant under
    # within-core local reordering, so counts only re-index
    rank2node = np.full(NRANK, -1, np.int64)
    rank2node[node2rank] = np.arange(N)
    new_node2rank = np.empty(N, np.int64)
    for c in range(NC):
        o = np.argsort(-counts[c, 0, :NPC_REAL], kind="stable")
        olds = rank2node[c * NPC + o]
        assert (olds >= 0).all()
        new_node2rank[olds] = c * NPC + np.arange(NPC_REAL)
    node2rank = new_node2rank
    per_core, counts = shard(node2rank)

    perms = np.empty((NC, NB, NPC), np.int64)
    for c in range(NC):
        for b in range(NB):
            perms[c, b] = np.argsort(-counts[c, b], kind="stable")

    # unified k per (bucket, block) across cores
    kk = np.zeros((NB, NBLK), np.int64)
    for b in range(NB):
        cnt = np.take_along_axis(counts[:, b], perms[:, b], axis=1)
        kk[b] = cnt.reshape(NC, NBLK, P).max(axis=(0, 2))

    # compute groups: consecutive same-k blocks, m*k <= SBATCH
    # bucket 0 (identity perm) goes last so its SBUF partials feed the
    # combine directly
    BORD = [1, 2, 3, 0]
    groups = []  # (b, g0, m, k)
    for b in BORD:
        g = 0
        while g < NBLK:
            k = int(kk[b, g])
            if k == 0:
                g += 1
                continue
            mlim = max(1, SBATCH // k)
            m = 1
            while (m < mlim and g + m < NBLK and kk[b, g + m] == k):
                m += 1
            groups.append((b, g, m, k))
            g += m

    # sbatches: consecutive same-bucket groups, total slots <= SBATCH
    sbatches = []
    gi = 0
    while gi < len(groups):
        b = groups[gi][0]
        glist = []
        S = 0
        while gi < len(groups) and groups[gi][0] == b:
            _, g0, m, k = groups[gi]
            if S + m * k > SBATCH:
                break
            glist.append((g0, m, k, S))
            S += m * k
            gi += 1
        sbatches.append(dict(b=b, glist=glist, S=S))

    # per-core gather index streams, in sbatch/piece order
    gstream = [[] for _ in range(NC)]
    col = 0
    for sb in sbatches:
        b = sb["b"]
        rects = []
        for c in range(NC):
            s_c, d_c, b_c = per_core[c]
            mm = b_c == b
            sb_s, sb_d = s_c[mm], d_c[mm]
            o = np.argsort(sb_d, kind="stable")
            sb_s, sb_d = sb_s[o], sb_d[o]
            starts = np.searchsorted(sb_d, np.arange(NPC))
            ends = np.searchsorted(sb_d, np.arange(NPC) + 1)
            rect = np.full((sb["S"], P), NPC_REAL, np.int64)  # dummy row
            for (g0, m, k, off) in sb["glist"]:
                nodes = perms[c, b, g0 * P:(g0 + m) * P]
                for u in range(m):
                    nd = nodes[u * P:(u + 1) * P]
                    for p, nloc in enumerate(nd):
                        s0, s1 = starts[nloc], ends[nloc]
                        cnt = s1 - s0
                        row0 = off + u * k
                        vals = np.sort(sb_s[s0:s1] - BUCKET * b)
                        rect[row0:row0 + cnt, p] = vals
            rects.append(rect)
        pieces = []
        t0 = 0
        while t0 < sb["S"]:
            tp = min(CAPS, sb["S"] - t0)
            pieces.append((col, tp * 8, tp, t0))
            for c in range(NC):
                part = rects[c][t0:t0 + tp, :].reshape(-1)
                gstream[c].append(wrap16(part))
            col += tp * 8
            t0 += tp
        sb["pieces"] = pieces
    gidx_arr = [np.concatenate(gstream[c], axis=1) for c in range(NC)]

    # combine gather idxs: for final (p, g) the position of node g*128+p in
    # the bucket-b partial table (row = p_b*NBLK + g_b).
    cstream = [[] for _ in range(NC)]
    meta_c = {}   # bucket (1..3) -> list of (col0, cols, tp, t0)
    ccol = 0
    for b in (1, 2, 3):
        pieces = []
        t0 = 0
        while t0 < NBLK:
            tp = min(CAPS, NBLK - t0)
            pieces.append((ccol, tp * 8, tp, t0))
            ccol += tp * 8
            t0 += tp
        meta_c[b] = pieces
    for c in range(NC):
        for b in (1, 2, 3):
            inv = np.empty(NPC, np.int64)
            inv[perms[c, b]] = np.arange(NPC)
            pos = (inv % P) * NBLK + (inv // P)
            for (col0, cols, tp, t0) in meta_c[b]:
                part = pos[t0 * P:(t0 + tp) * P]
                cstream[c].append(wrap16(part))
    cidx_arr = [np.concatenate(cstream[c], axis=1) for c in range(NC)]

    # a_d idx stream: per bucket, perm order (local ranks), pieces of CAPS
    adstream = [[] for _ in range(NC)]
    meta_ad = {}
    acol = 0
    for b in (1, 2, 3):
        pieces = []
        t0 = 0
        while t0 < NBLK:
            tp = min(CAPS, NBLK - t0)
            pieces.append((acol, tp * 8, tp, t0))
            for c in range(NC):
                part = perms[c, b][t0 * P:(t0 + tp) * P]
                adstream[c].append(wrap16(part))
            acol += tp * 8
            t0 += tp
        meta_ad[b] = pieces
    adidx_arr = [np.concatenate(adstream[c], axis=1) for c in range(NC)]

    return dict(node2rank=node2rank, sbatches=sbatches, meta_c=meta_c,
                meta_ad=meta_ad, gidx=gidx_arr, cidx=cidx_arr,
                adidx=adidx_arr, perms=perms, gcols=col, ccols=ccol,
                adcols=acol)


# ------------------------------------------------------- program builder
def build_program(prep, weights):
    sbatches = prep["sbatches"]
    meta_c, meta_ad = prep["meta_c"], prep["meta_ad"]
    b1 = weights["b1"]; b2 = weights["b2"]
    bl = float(weights["bl"][0])
    if np.abs(b1).max() > 0 or np.abs(b2).max() > 0:
        raise NotImplementedError("nonzero b1/b2")

    nc = bacc.Bacc("TRN2", target_bir_lowering=False, debug=False,
                   enable_asserts=False, num_devices=NC,
                   num_swdge_queues=4,
                   dynamic_dma_scratch_size=32768)

    # ---- external tensors
    adc1 = nc.dram_tensor("adc1", [P, NB, NBLK], DT, kind="ExternalInput")
    selfc1_d = nc.dram_tensor("selfc1", [P, NBLK, L1W], DT, kind="ExternalInput")
    gidx_d = nc.dram_tensor("gidx", [P, prep["gcols"]], DI, kind="ExternalInput")
    cidx_d = nc.dram_tensor("cidx", [P, prep["ccols"]], DI, kind="ExternalInput")
    adidx_d = nc.dram_tensor("adidx", [P, prep["adcols"]], DI, kind="ExternalInput")
    consts = nc.dram_tensor("consts", [P, 768], DT, kind="ExternalInput")
    y_d = nc.dram_tensor("y", [NPC, 1], DT, kind="ExternalOutput")

    # ---- internal DRAM
    tab1 = nc.dram_tensor("tab1", [NRANK, ROWF], DT, kind="ExternalInput")
    agin2 = nc.dram_tensor("agin2", [NPC, ROWH], DH)
    tab2 = nc.dram_tensor("tab2", [NRANK, ROWH], DH, addr_space="Shared")
    # per-bucket partial tables, 256B rows, row index = p*NBLK + g
    part_t = [[nc.dram_tensor(f"part{li}_{b}", [NPC, ROWH], DH)
               for b in range(NB)] for li in range(2)]

    with tile.TileContext(nc) as tc:
        with tc.tile_pool(name="const", bufs=1) as cpool, \
             tc.tile_pool(name="chunk", bufs=2) as chpool, \
             tc.tile_pool(name="small", bufs=3) as zpool, \
             tc.tile_pool(name="gix", bufs=2) as gixpool, \
             tc.tile_pool(name="pb", bufs=2) as pbpool, \
             tc.tile_pool(name="cg", bufs=4) as cgpool, \
             tc.tile_pool(name="psum", bufs=2, space="PSUM") as pspool:

            ct = cpool.tile([P, 768], DT)
            nc.sync.dma_start(ct[:], consts[:])
            W1diag = ct[:, 0:128]      # valid on partitions 0:12
            W2diag = ct[:, 128:256]
            vs2bc = ct[:, 256:288]
            vd2bc = ct[:, 288:320]
            Wlbc = ct[:, 320:352]
            ident = ct[:, 352:480]
            padmaskh = ct[:, 640:640 + NBLK]

            adcol1 = cpool.tile([P, NB, NBLK], DT, tag="adcol1")
            nc.sync.dma_start(adcol1[:], adc1[:])
            adcol2 = cpool.tile([P, NB, NBLK], DH, tag="adcol2")
            selfc1 = cpool.tile([P, NBLK, L1W], DT, tag="selfc1")
            nc.sync.dma_start(selfc1[:], selfc1_d[:])
            selft2 = cpool.tile([P, NBLK, L2W], DH, tag="selft2")
            cixt = cpool.tile([P, prep["ccols"]], DI, tag="cixt")
            nc.sync.dma_start(cixt[:], cidx_d[:])
            adixt = cpool.tile([P, prep["adcols"]], DI, tag="adixt")
            nc.sync.dma_start(adixt[:], adidx_d[:])

            qrr = [0]

            def nextq():
                qrr[0] = (qrr[0] + 1) % 4
                return qrr[0]

            def cgather(li, b, W):
                cg = cgpool.tile([P, NBLK, W], DH, tag="cg")
                for (col0, cols, tp, t0) in meta_c[b]:
                    dma_gather_raw(
                        nc.gpsimd, cg[:, t0:t0 + tp, :],
                        part_t[li][b][:, 0:W],
                        cixt[:, col0:col0 + cols], tp * P, W, ROWH,
                        queue_num=nextq())
                return cg

            def edge_phase(li, tab, W, adcol_fn, dt_row=DT, estep=ROWF,
                           slab0=None):
                slab = {"tile": slab0, "base": 0 if slab0 is not None else -1}

                def gix(col0, cols):
                    if (slab["tile"] is None or col0 < slab["base"]
                            or col0 + cols > slab["base"] + GSLAB):
                        t = gixpool.tile([P, GSLAB], DI, tag="gslab")
                        base = col0
                        csz = min(GSLAB, prep["gcols"] - base)
                        nc.sync.dma_start(t[:, 0:csz], gidx_d[:, base:base + csz])
                        slab["tile"], slab["base"] = t, base
                    b0 = col0 - slab["base"]
                    return slab["tile"][:, b0:b0 + cols]

                cgs = {}
                pb = None
                prev_b = -1
                pend = None   # (pb_tile, bucket) awaiting store

                def flush(pend_):
                    pb_, b_ = pend_
                    dest = part_t[li][b_][:].rearrange(
                        "(p g) w -> p g w", p=P)[:, :, 0:W]
                    nc.sync.dma_start(dest, pb_[:])

                for sb in sbatches:
                    b = sb["b"]
                    if b != prev_b:
                        if pb is not None:
                            assert pend is None
                            pend = (pb, prev_b)
                        pb = pbpool.tile([P, NBLK, W], DH, tag="pb")
                        nc.vector.memset(pb[:], 0.0)
                        prev_b = b
                        assert b != 0 or sb is sbatches[-1] or True
                    S = sb["S"]
                    chunk = chpool.tile([P, SBATCH, W], dt_row, tag="chunk")
                    for (col0, cols, tp, t0) in sb["pieces"]:
                        dma_gather_raw(
                            nc.gpsimd, chunk[:, t0:t0 + tp, :],
                            tab[BUCKET * b:BUCKET * (b + 1), 0:W],
                            gix(col0, cols), tp * P, W, estep,
                            queue_num=nextq())
                    if pend is not None:
                        # software pipeline: the store of the previous
                        # bucket's partials goes out after this sbatch's
                        # gathers; its combine gather follows
                        flush(pend)
                        cgs[pend[1]] = cgather(li, pend[1], W)
                        pend = None
                    z = zpool.tile([P, SBATCH], DT, tag="z")
                    ad = adcol_fn(b)
                    for (g0, m, k, off) in sb["glist"]:
                        nc.vector.tensor_tensor(
                            out=z[:, off:off + m * k].rearrange(
                                "p (m k) -> p m k", m=m),
                            in0=chunk[:, off:off + m * k, W - 1].rearrange(
                                "p (m k) -> p m k", m=m),
                            in1=ad[:, g0:g0 + m].rearrange(
                                "p (m o) -> p m o", o=1).to_broadcast([P, m, k]),
                            op=mybir.AluOpType.add)
                    zp = zpool.tile([P, SBATCH], DT, tag="zp")
                    nc.scalar.activation(zp[:, 0:S], z[:, 0:S],
                                         mybir.ActivationFunctionType.Prelu,
                                         alpha=NEG)
                    ex = zpool.tile([P, SBATCH], DT, tag="ex")
                    nc.scalar.activation(ex[:, 0:S], zp[:, 0:S],
                                         mybir.ActivationFunctionType.Exp)
                    nc.vector.tensor_tensor(
                        out=chunk[:, 0:S, 0:W - 1],
                        in0=chunk[:, 0:S, 0:W - 1],
                        in1=ex[:, 0:S].to_broadcast([P, S, W - 1]),
                        op=mybir.AluOpType.mult)
                    with nc.allow_low_precision(
                            reason="fp16 partials; DVE accumulates f32 "
                                   "internally, rel tol 2e-2"):
                        for (g0, m, k, off) in sb["glist"]:
                            nc.vector.tensor_reduce(
                                out=pb[:, g0:g0 + m, 0:W - 1],
                                in_=chunk[:, off:off + m * k, 0:W - 1].rearrange(
                                    "p (m k) w -> p m w k", m=m),
                                axis=mybir.AxisListType.X,
                                op=mybir.AluOpType.add)
                            nc.vector.tensor_reduce(
                                out=pb[:, g0:g0 + m, W - 1],
                                in_=ex[:, off:off + m * k].rearrange(
                                    "p (m k) -> p m k", m=m),
                                axis=mybir.AxisListType.X,
                                op=mybir.AluOpType.add)
                # bucket 0 is processed last with an identity perm: its
                # SBUF partials feed the combine directly
                assert prev_b == 0
                cgs[0] = pb
                return cgs

            def combine(cgs, W, selftile, spool, tag):
                v = spool.tile([P, NBLK, W], DT, tag=tag)
                nc.vector.tensor_tensor(out=v[:], in0=cgs[0][:], in1=cgs[1][:],
                                        op=mybir.AluOpType.add)
                nc.vector.tensor_tensor(out=v[:], in0=v[:], in1=cgs[2][:],
                                        op=mybir.AluOpType.add)
                nc.vector.tensor_tensor(out=v[:], in0=v[:], in1=cgs[3][:],
                                        op=mybir.AluOpType.add)
                # self-loop term: [values 0:W-1, den at W-1]
                nc.vector.tensor_tensor(out=v[:, :, 0:W], in0=v[:, :, 0:W],
                                        in1=selftile[:],
                                        op=mybir.AluOpType.add)
                return v

            # ---------------- layer 1
            cgs1 = edge_phase(0, tab1, L1W, lambda b: adcol1[:, b, :])

            f1pool = tc.tile_pool(name="f1", bufs=1)
            spool = f1pool.__enter__()
            v1 = combine(cgs1, L1W, selfc1, spool, "v1")
            rec1 = spool.tile([P, NBLK], DT, tag="rec1")
            nc.vector.tensor_scalar_add(rec1[:], v1[:, :, L1W - 1], EPS)
            nc.vector.reciprocal(rec1[:], rec1[:])
            vst1 = spool.tile([P, NBLK, 3], DT, tag="vst1")
            nc.vector.tensor_tensor(out=vst1[:], in0=v1[:, :, 0:3],
                                    in1=rec1[:].to_broadcast([P, NBLK, 3]),
                                    op=mybir.AluOpType.mult)
            # finalize-1 in halves of 49 blocks: W1 sandwich, a_s2/a_d2,
            # self-term, agin2 store per half (overlaps compute / DMA)
            for h in range(2):
                u0, u1 = h * HB, (h + 1) * HB
                sth = spool.tile([P, HB, ROWH], DH, tag="sth")
                for u in range(u0, u1, 4):
                    nu = min(4, u1 - u)
                    tp1 = pspool.tile([3 * nu, P], DT, space="PSUM", tag="tps")
                    nc.tensor.transpose(
                        out=tp1[:],
                        in_=vst1[:, u:u + nu, :].rearrange("p a b -> p (a b)"),
                        identity=ident[:])
                    t1s = zpool.tile([3 * nu, P], DT, tag="t1s")
                    nc.vector.tensor_copy(out=t1s[:], in_=tp1[:])
                    hp = pspool.tile([P, nu * 32], DT, space="PSUM", tag="hps")
                    nc.tensor.matmul(hp[:], t1s[:],
                                     W1diag[0:3 * nu, 0:nu * 32],
                                     start=True, stop=True)
                    nc.scalar.activation(
                        sth[:, u - u0:u - u0 + nu, 0:32],
                        hp[:].rearrange("p (a b) -> p a b", a=nu),
                        mybir.ActivationFunctionType.Relu)
                tmp2 = spool.tile([P, HB, 32], DT, tag="tmp2")
                asd = spool.tile([P, HB], DT, tag="asd")
                # a_s2 (+ pad mask)
                nc.vector.tensor_tensor(
                    out=tmp2[:], in0=sth[:, :, 0:32],
                    in1=vs2bc.rearrange("p (o w) -> p o w", o=1).to_broadcast(
                        [P, HB, 32]),
                    op=mybir.AluOpType.mult)
                nc.vector.tensor_reduce(out=asd[:], in_=tmp2[:],
                                        axis=mybir.AxisListType.X,
                                        op=mybir.AluOpType.add)
                nc.vector.tensor_tensor(out=sth[:, :, 32], in0=asd[:],
                                        in1=padmaskh[:, u0:u1],
                                        op=mybir.AluOpType.add)
                # a_d2
                nc.vector.tensor_tensor(
                    out=tmp2[:], in0=sth[:, :, 0:32],
                    in1=vd2bc.rearrange("p (o w) -> p o w", o=1).to_broadcast(
                        [P, HB, 32]),
                    op=mybir.AluOpType.mult)
                nc.vector.tensor_reduce(out=asd[:], in_=tmp2[:],
                                        axis=mybir.AxisListType.X,
                                        op=mybir.AluOpType.add)
                nc.vector.tensor_copy(out=sth[:, :, 33], in_=asd[:])
                nc.vector.tensor_copy(out=adcol2[:, 0, u0:u1], in_=asd[:])
                nc.vector.memset(sth[:, :, 34:ROWH], 0.0)
                # layer-2 self term: es2 = exp(prelu(a_s2+pad + a_d2))
                zs = zpool.tile([P, HB], DT, tag="zs")
                nc.vector.tensor_tensor(out=zs[:], in0=sth[:, :, 32],
                                        in1=asd[:], op=mybir.AluOpType.add)
                nc.scalar.activation(zs[:], zs[:],
                                     mybir.ActivationFunctionType.Prelu,
                                     alpha=NEG)
                nc.scalar.activation(zs[:], zs[:],
                                     mybir.ActivationFunctionType.Exp)
                nc.vector.tensor_tensor(
                    out=selft2[:, u0:u1, 0:32], in0=sth[:, :, 0:32],
                    in1=zs[:].to_broadcast([P, HB, 32]),
                    op=mybir.AluOpType.mult)
                nc.vector.tensor_copy(out=selft2[:, u0:u1, 32], in_=zs[:])
                nc.scalar.dma_start(
                    agin2[:].rearrange("(g p) w -> p g w", p=P)[:, u0:u1, :],
                    sth[:])
            # start the exchange first, then the local a_d2 gathers
            # (buckets 1-3; bucket 0 was copied directly from asd) overlap
            # the transfer
            nc.gpsimd.collective_compute(
                "AllGather", mybir.AluOpType.bypass,
                replica_groups=[list(range(NC))],
                ins=[agin2[:]], outs=[tab2[:]])
            for b in (1, 2, 3):
                for (col0, cols, tp, t0) in meta_ad[b]:
                    dma_gather_raw(
                        nc.gpsimd,
                        adcol2[:, b, t0:t0 + tp].rearrange("p (g o) -> p g o", o=1),
                        agin2[:, 33:34], adixt[:, col0:col0 + cols],
                        tp * P, 1, ROWH, queue_num=nextq())
            # preload layer-2's first gidx slab while the AllGather flies
            slab2 = gixpool.tile([P, GSLAB], DI, tag="gslab")
            nc.sync.dma_start(slab2[:], gidx_d[:, 0:GSLAB])

            f1pool.__exit__(None, None, None)

            # ---------------- layer 2
            cgs2 = edge_phase(1, tab2, L2W, lambda b: adcol2[:, b, :],
                              dt_row=DH, estep=ROWH, slab0=slab2)

            f2pool = tc.tile_pool(name="f2", bufs=1)
            spool = f2pool.__enter__()
            v2 = combine(cgs2, L2W, selft2, spool, "v2")
            rec2 = spool.tile([P, NBLK], DT, tag="rec2")
            nc.vector.tensor_scalar_add(rec2[:], v2[:, :, 32], EPS)
            nc.vector.reciprocal(rec2[:], rec2[:])
            vst2 = spool.tile([P, NBLK, 32], DT, tag="vst2")
            nc.vector.tensor_tensor(out=vst2[:], in0=v2[:, :, 0:32],
                                    in1=rec2[:].to_broadcast([P, NBLK, 32]),
                                    op=mybir.AluOpType.mult)
            hf = spool.tile([P, NBLK, 32], DT, tag="hf")
            for u in range(0, NBLK, 4):
                nu = min(4, NBLK - u)
                tp2 = pspool.tile([32 * nu, P], DT, space="PSUM", tag="tps")
                nc.tensor.transpose(
                    out=tp2[:],
                    in_=vst2[:, u:u + nu, :].rearrange("p a b -> p (a b)"),
                    identity=ident[:])
                t2s = zpool.tile([32 * nu, P], DT, tag="t2s")
                nc.vector.tensor_copy(out=t2s[:], in_=tp2[:])
                hp2 = pspool.tile([P, nu * 32], DT, space="PSUM", tag="hps")
                nc.tensor.matmul(hp2[:], t2s[:], W2diag[0:32 * nu, 0:nu * 32],
                                 start=True, stop=True)
                nc.scalar.activation(
                    hf[:, u:u + nu, :],
                    hp2[:].rearrange("p (a b) -> p a b", a=nu),
                    mybir.ActivationFunctionType.Relu)
            # reuse vst2 as scratch for the Wl product (hf is final by now)
            nc.vector.tensor_tensor(out=vst2[:], in0=hf[:],
                                    in1=Wlbc.rearrange("p (o w) -> p o w", o=1).to_broadcast([P, NBLK, 32]),
                                    op=mybir.AluOpType.mult)
            ycol = spool.tile([P, NBLK], DT, tag="ycol")
            nc.vector.tensor_reduce(out=ycol[:], in_=vst2[:],
                                    axis=mybir.AxisListType.X,
                                    op=mybir.AluOpType.add)
            if bl != 0.0:
                nc.vector.tensor_scalar_add(ycol[:], ycol[:], bl)
            nc.scalar.dma_start(
                y_d[:].rearrange("(g p) w -> p (g w)", p=P), ycol[:])
            f2pool.__exit__(None, None, None)

    nc.compile()
    return nc


def build_consts(weights):
    W1 = weights["W1"].astype(np.float32)
    W2 = weights["W2"].astype(np.float32)
    vs2 = (W2 @ weights["att_src2"]).astype(np.float32)
    vd2 = (W2 @ weights["att_dst2"]).astype(np.float32)
    Wl = weights["Wl"][:, 0].astype(np.float32)
    ct = np.zeros((P, 768), np.float32)
    for u in range(4):
        ct[3 * u:3 * u + 3, 0 + 32 * u:0 + 32 * u + 32] = W1
    for u in range(4):
        ct[32 * u:32 * u + 32, 128 + 32 * u:128 + 32 * u + 32] = W2
    ct[:, 256:288] = vs2[None, :]
    ct[:, 288:320] = vd2[None, :]
    ct[:, 320:352] = Wl[None, :]
    ct[:, 352:480] = np.eye(P, dtype=np.float32)
    pmh = np.zeros((P, NBLK), np.float32)
    pmh[84:128, NBLK - 1] = -30000.0
    ct[:, 640:640 + NBLK] = pmh
    return ct


def build_inputs(x, prep, weights):
    node2rank = prep["node2rank"]
    xr = np.zeros((NRANK, 3), np.float32)
    xr[node2rank] = x
    vs1 = (weights["W1"] @ weights["att_src1"]).astype(np.float32)
    vd1 = (weights["W1"] @ weights["att_dst1"]).astype(np.float32)
    a_s1 = xr @ vs1
    a_d1 = xr @ vd1
    pad = np.arange(NRANK).reshape(NC, NPC)[:, NPC_REAL:].ravel()
    a_s1[pad] = A_S_PAD
    tab1 = np.zeros((NRANK, ROWF), np.float32)
    tab1[:, 0:3] = xr
    tab1[:, 3] = a_s1
    # layer-1 self-loop term per node (rank order): exp(prelu(as+ad))*[x, 1]
    zs = a_s1 + a_d1
    es = np.exp(np.where(zs >= 0, zs, NEG * zs)).astype(np.float32)
    es[pad] = 0.0
    selfc = np.concatenate([xr * es[:, None], es[:, None]], axis=1)
    ct = build_consts(weights)
    per_core = []
    for c in range(NC):
        adl = a_d1[c * NPC:(c + 1) * NPC]
        adc = np.zeros((P, NB, NBLK), np.float32)
        for b in range(NB):
            perm = prep["perms"][c, b]
            adc[:, b] = adl[perm].reshape(NBLK, P).T
        sc = selfc[c * NPC:(c + 1) * NPC].reshape(NBLK, P, L1W).transpose(1, 0, 2)
        per_core.append({
            "tab1": tab1, "adc1": adc, "selfc1": np.ascontiguousarray(sc),
            "gidx": prep["gidx"][c], "cidx": prep["cidx"][c],
            "adidx": prep["adidx"][c], "consts": ct,
        })
    return per_core


_CACHE = {}
LAST_EXEC_NS = None
LAST_RESULTS = None


def kernel(**inputs):
    x = np.asarray(inputs["x"], np.float32)
    edge_index = np.asarray(inputs["edge_index"])
    weights = {k: np.asarray(v, np.float32) for k, v in inputs.items()
               if k not in ("x", "edge_index")}

    key = edge_index.tobytes()[:64]  # cheap cache key
    if key not in _CACHE:
        prep = preprocess(edge_index)
        nc = build_program(prep, weights)
        _CACHE[key] = (prep, nc)
    prep, nc = _CACHE[key]

    in_maps = build_inputs(x, prep, weights)
    import os
    trace = bool(int(os.environ.get("GAT_TRACE", "0")))
    res = run_bass_kernel_spmd(nc, in_maps, core_ids=list(range(NC)),
                               trace=trace)
    global LAST_EXEC_NS, LAST_RESULTS
    LAST_EXEC_NS = res.exec_time_ns
    LAST_RESULTS = res
    y = np.zeros((N, 1), np.float32)
    yr = np.concatenate([res.results[c]["y"] for c in range(NC)], axis=0)
    y[:, 0] = yr[prep["node2rank"], 0]
    return y


if __name__ == "__main__":
    d = np.load("/root/problem/work/inputs.npz")
    inp = {k: d[k] for k in d.files}
    y = kernel(**inp)
    y_ref = np.load("/root/problem/work/y_ref.npy")
    rel = np.abs(y - y_ref).max() / np.abs(y_ref).max()
    print("rel err:", rel)
